# revision 1
# baseline (speedup 1.0000x reference)
"""Causal multi-head attention (B=128, T=256, C=384, H=6, Dh=64) on 8 TRN2
NeuronCores, data-parallel over batch (16 batches per core, no collectives).

Layout strategy per core:
  - host pre-transposes x to xT [b, C, T] and casts activations/weights to bf16
  - QT/KT computed as [D, T] (Dh on partitions) so scores = QT_h.T @ KT_h needs
    no on-chip transpose of Q/K
  - V computed as [T, D] so AV contraction (over key positions) has keys on
    partitions
  - softmax over the free dim (keys) without max-subtraction (scores are
    O(10) here, exp cannot overflow in fp32); row sums fused into the exp
    activation via accum_out
  - P is transposed on the PE (bf16, 1 cycle/row) for the AV matmul
  - output projection consumes OT [D, T] as the stationary operand directly
"""

import sys

sys.path.insert(0, "/opt/trn_rl_repo")

import numpy as np
import ml_dtypes

import concourse.bass as bass
import concourse.tile as tile
from concourse import mybir
from concourse.bass_utils import run_bass_kernel_spmd
from concourse.masks import make_causal_mask, make_identity

def split_multi_waits(nc):
    """This walrus build accepts at most one sync-wait command per
    instruction; hoist extra waits into standalone InstEventSemaphore
    instructions on the same engine queue (queue waits run in order before
    the original instruction, so semantics are preserved)."""
    ctr = [0]

    def mk(engine, wait):
        ctr[0] += 1
        return mybir.InstEventSemaphore(
            name=f"WSPLIT-{ctr[0]}",
            engine=engine,
            ins=[],
            outs=[],
            sync_info=mybir.SyncInfo(on_wait=[wait], on_update=[]),
        )

    for f in nc.m.functions:
        for blk in f.blocks:
            insts = blk.instructions
            out = []
            for inst in insts:
                si = inst.sync_info
                if si is not None and len(si.on_wait) > 1:
                    waits = list(si.on_wait)
                    for w in waits[:-1]:
                        out.append(mk(inst.engine, w))
                    inst.sync_info = mybir.SyncInfo(
                        on_wait=[waits[-1]], on_update=list(si.on_update)
                    )
                out.append(inst)
            insts[:] = out
    return nc


N_CORES = 8
B, T, C = 128, 256, 384
H, DH = 6, 64
BL = B // N_CORES  # batches per core
BF16 = mybir.dt.bfloat16
FP32 = mybir.dt.float32
AFT = mybir.ActivationFunctionType
SCALE = DH**-0.5  # 0.125
NEG = -1.0e9


def build_kernel() -> bass.Bass:
    nc = bass.Bass()
    xT = nc.dram_tensor("xT", [BL, C, T], BF16, kind="ExternalInput")
    wqt = nc.dram_tensor("wqt", [C, C], BF16, kind="ExternalInput")  # Wq.T [C, D]
    wkt = nc.dram_tensor("wkt", [C, C], BF16, kind="ExternalInput")
    wvt = nc.dram_tensor("wvt", [C, C], BF16, kind="ExternalInput")
    wot = nc.dram_tensor("wot", [C, C], BF16, kind="ExternalInput")  # Wo.T [D, C]
    y = nc.dram_tensor("y", [BL, T, C], FP32, kind="ExternalOutput")

    GB = 2  # batches per projection group (N = GB*T = 512 <= one PSUM bank fp32)
    with tile.TileContext(nc) as tc:
        with (
            tc.tile_pool(name="const", bufs=1) as const,
            tc.tile_pool(name="xp", bufs=2) as xp,
            tc.tile_pool(name="qkv", bufs=2) as qkv,
            tc.tile_pool(name="pp", bufs=3) as pp,
            tc.tile_pool(name="ptp", bufs=3) as ptp,
            tc.tile_pool(name="st", bufs=4) as st,
            tc.tile_pool(name="otp", bufs=2) as otp,
            tc.tile_pool(name="yp", bufs=3) as yp,
            tc.tile_pool(name="psA", bufs=6, space="PSUM") as psA,
            tc.tile_pool(name="psO", bufs=2, space="PSUM") as psO,
        ):
            ident = const.tile([128, 128], BF16)
            make_identity(nc, ident)
            # multiplicative 0/1 causal masks (bf16), applied post-exp
            m0 = const.tile([128, 128], BF16)
            nc.gpsimd.memset(m0, 1.0)
            nc.gpsimd.affine_select(
                out=m0, in_=m0, compare_op=mybir.AluOpType.is_ge,
                fill=0.0, base=0, pattern=[[-1, 128]], channel_multiplier=1,
            )
            # combined per-head mask over [tq0 keys 0:128 | tq1 keys 0:256]
            mc = const.tile([128, 384], BF16)
            nc.gpsimd.memset(mc, 1.0)
            nc.vector.tensor_copy(mc[:, 0:128], m0)
            nc.vector.tensor_copy(mc[:, 256:384], m0)

            w_sb = {}
            for name, dram in (("wq", wqt), ("wk", wkt), ("wv", wvt), ("wo", wot)):
                w = const.tile([128, 3, C], BF16, tag=name)
                nc.sync.dma_start(out=w, in_=dram.rearrange("(k p) d -> p k d", p=128))
                w_sb[name] = w

            for g in range(BL // GB):
                # ---- load xT for GB batches: [128, k, b, T] ----
                xt = xp.tile([128, 3, GB, T], BF16)
                for bi in range(GB):
                    nc.sync.dma_start(
                        out=xt[:, :, bi, :],
                        in_=xT[g * GB + bi].rearrange("(k p) t -> p k t", p=128),
                    )

                # ---- QT/KT for both batches: [D, b, T], N = GB*T ----
                qt = qkv.tile([128, 3, GB, T], BF16, tag="qt")
                kt = qkv.tile([128, 3, GB, T], BF16, tag="kt")
                for dst, wname in ((qt, "wq"), (kt, "wk")):
                    w = w_sb[wname]
                    for d in range(3):
                        ps = psA.tile([128, GB * T], FP32, tag="big")
                        for k in range(3):
                            nc.tensor.matmul(
                                ps,
                                lhsT=w[:, k, d * 128 : (d + 1) * 128],
                                rhs=xt[:, k, :, :],
                                start=(k == 0),
                                stop=(k == 2),
                            )
                        nc.any.tensor_copy(dst[:, d, :, :], ps)

                # ---- V = [T, D] per batch ----
                vs = []
                for bi in range(GB):
                    v = qkv.tile([128, 2, C], BF16, tag=f"v{bi}")
                    for t2 in range(2):
                        ps = psA.tile([128, GB * T], FP32, tag="big")
                        for k in range(3):
                            nc.tensor.matmul(
                                ps[:, 0:C],
                                lhsT=xt[:, k, bi, t2 * 128 : (t2 + 1) * 128],
                                rhs=w_sb["wv"][:, k, :],
                                start=(k == 0),
                                stop=(k == 2),
                            )
                        nc.any.tensor_copy(v[:, t2, :], ps[:, 0:C])
                    vs.append(v)

                for bi in range(GB):
                    b = g * GB + bi
                    v = vs[bi]
                    # ---- attention, one head-pair at a time ----
                    ot = otp.tile([128, 3, T], BF16)  # OT [D, T]
                    for pair in range(3):
                        po = psO.tile([128, T], FP32)
                        # scores: row-packed pair (even head rows 0:64, odd
                        # 64:128); per head one psum tile [128, 384] holding
                        # [tq0 x ts0:128 | tq1 x ts0:256]
                        sc = [None, None]
                        for sub in range(2):
                            sc[sub] = psA.tile(
                                [128, 384], FP32, tag="big", name=f"sc{sub}"
                            )
                        for blkid in range(2):
                            for sub in range(2):
                                doff = sub * 64
                                qh = qt[doff : doff + 64, pair, bi, :]
                                kh = kt[doff : doff + 64, pair, bi, :]
                                if blkid == 0:
                                    nc.tensor.matmul(
                                        sc[sub][:, 0:128],
                                        lhsT=qh[:, 0:128], rhs=kh[:, 0:128],
                                        start=True, stop=True,
                                    )
                                else:
                                    nc.tensor.matmul(
                                        sc[sub][:, 128:384],
                                        lhsT=qh[:, 128:T], rhs=kh,
                                        start=True, stop=True,
                                    )
                        # softmax: exp straight from PSUM (no max-subtraction),
                        # multiplicative causal mask, then row-normalize
                        ps_ = []
                        for sub in range(2):
                            p = pp.tile([128, 384], BF16, tag=f"p{sub}")
                            sums = st.tile([128, 2], FP32, tag=f"sums{sub}")
                            rs = st.tile([128, 2], FP32, tag=f"rs{sub}")
                            nc.scalar.activation(p, sc[sub], AFT.Exp, scale=SCALE)
                            nc.vector.tensor_mul(p, p, mc)
                            # partition p holds two queries: tq0 row p in cols
                            # 0:128 and tq1 row p in cols 128:384 — separate sums
                            nc.vector.reduce_sum(
                                out=sums[:, 0:1], in_=p[:, 0:128],
                                axis=mybir.AxisListType.X,
                            )
                            nc.vector.reduce_sum(
                                out=sums[:, 1:2], in_=p[:, 128:384],
                                axis=mybir.AxisListType.X,
                            )
                            nc.vector.reciprocal(rs, sums)
                            nc.vector.tensor_scalar_mul(
                                p[:, 0:128], p[:, 0:128], rs[:, 0:1]
                            )
                            nc.vector.tensor_scalar_mul(
                                p[:, 128:384], p[:, 128:384], rs[:, 1:2]
                            )
                            ps_.append(p)
                        # transpose P blocks on the PE: PT[ts, tq]
                        pts = []
                        for sub in range(2):
                            p = ps_[sub]
                            pt = ptp.tile([128, 2, T], BF16, tag=f"pt{sub}")
                            tp = psA.tile([128, T], BF16, tag="big", name="tp")
                            nc.tensor.transpose(tp[:, 0:128], p[:, 0:128], ident)
                            nc.tensor.transpose(tp[:, 128:T], p[:, 128:256], ident)
                            nc.any.tensor_copy(pt[:, 0, :], tp)
                            tp2 = psA.tile([128, 128], BF16, tag="big", name="tp2")
                            nc.tensor.transpose(tp2, p[:, 256:384], ident)
                            nc.any.tensor_copy(pt[:, 1, 128:T], tp2)
                            pts.append(pt)
                        # AV: col-packed pair; interleave even/odd for overlap
                        for mm in range(3):
                            for sub in range(2):
                                h = 2 * pair + sub
                                doff = sub * 64
                                pt = pts[sub]
                                out_ap = po[doff : doff + 64, :]
                                if mm == 0:
                                    nc.tensor.matmul(
                                        out_ap[:, 0:128],
                                        lhsT=v[:, 0, h * 64 : (h + 1) * 64],
                                        rhs=pt[:, 0, 0:128],
                                        start=True, stop=True,
                                        tile_position=(0, doff),
                                    )
                                else:
                                    ts_ = mm - 1
                                    nc.tensor.matmul(
                                        out_ap[:, 128:T],
                                        lhsT=v[:, ts_, h * 64 : (h + 1) * 64],
                                        rhs=pt[:, ts_, 128:T],
                                        start=(ts_ == 0), stop=(ts_ == 1),
                                        tile_position=(0, doff),
                                    )
                        nc.any.tensor_copy(ot[:, pair, :], po)

                    # ---- y = OT.T @ WoT : [T, C] ----
                    for t2 in range(2):
                        ps = psA.tile([128, GB * T], FP32, tag="big")
                        for k in range(3):
                            nc.tensor.matmul(
                                ps[:, 0:C],
                                lhsT=ot[:, k, t2 * 128 : (t2 + 1) * 128],
                                rhs=w_sb["wo"][:, k, :],
                                start=(k == 0),
                                stop=(k == 2),
                            )
                        ys = yp.tile([128, C], FP32)
                        nc.any.tensor_copy(ys, ps[:, 0:C])
                        nc.sync.dma_start(
                            out=y[b, t2 * 128 : (t2 + 1) * 128, :], in_=ys
                        )
    return nc


_NC = None


def _get_nc():
    global _NC
    if _NC is None:
        _NC = split_multi_waits(build_kernel())
    return _NC


def kernel(x, Wq, Wk, Wv, Wo, _trace=False):
    bf16 = ml_dtypes.bfloat16
    wq_t = np.ascontiguousarray(Wq.T).astype(bf16)
    wk_t = np.ascontiguousarray(Wk.T).astype(bf16)
    wv_t = np.ascontiguousarray(Wv.T).astype(bf16)
    wo_t = np.ascontiguousarray(Wo.T).astype(bf16)
    in_maps = []
    for i in range(N_CORES):
        xs = x[i * BL : (i + 1) * BL]  # [BL, T, C]
        xs_t = np.ascontiguousarray(xs.transpose(0, 2, 1)).astype(bf16)
        in_maps.append(
            {"xT": xs_t, "wqt": wq_t, "wkt": wk_t, "wvt": wv_t, "wot": wo_t}
        )
    res = run_bass_kernel_spmd(
        _get_nc(), in_maps, list(range(N_CORES)), trace=_trace
    )
    out = np.concatenate([r["y"] for r in res.results], axis=0)
    if _trace:
        return out.astype(np.float32), res
    return out.astype(np.float32)



# revision 6
# speedup vs baseline: 1.6203x; 1.6203x over previous
"""Causal multi-head attention (B=128, T=256, C=384, H=6, Dh=64) on 8 TRN2
NeuronCores, data-parallel over batch (16 batches per core, no collectives).

Layout strategy per core (v2 — transposed-scores):
  - host pre-transposes x to xT [b, C, T] and casts activations/weights to bf16
  - QT/KT computed as [D, T] (Dh on partitions)
  - scores are computed TRANSPOSED: ST[ts, tq] = KT_h^T @ QT_h, so
    PT = exp(ST)*mask feeds the AV matmul directly as the stationary
    operand — no PE transposes of P at all
  - V stored per head with an appended ones column ([ts, 65]); the AV
    matmul out = PT^T @ [V_h | 1] lands O[tq, 0:64] AND the softmax
    denominators in col 64 of the same PSUM tile — row sums are free
  - normalization is a per-partition tensor_scalar_mul fused into the
    PSUM->SBUF move (exp is never max-subtracted; scores are O(30) so
    fp32 exp cannot overflow)
  - O [tq, D] is transposed on the PE (6 [128,128] blocks per batch) to
    OT for the output projection, which consumes OT as stationary
  - attention is software-pipelined one head ahead so the PE never waits
    on the scalar(exp)/vector(mask) stages of the same head
"""

import sys

sys.path.insert(0, "/opt/trn_rl_repo")

import numpy as np
import ml_dtypes

import concourse.bass as bass
import concourse.tile as tile
from concourse import mybir
from concourse.bass_utils import run_bass_kernel_spmd
from concourse.masks import make_identity


def split_multi_waits(nc):
    """This walrus build accepts at most one sync-wait command per
    instruction; hoist extra waits into standalone InstEventSemaphore
    instructions on the same engine queue (queue waits run in order before
    the original instruction, so semantics are preserved)."""
    ctr = [0]

    def mk(engine, wait):
        ctr[0] += 1
        return mybir.InstEventSemaphore(
            name=f"WSPLIT-{ctr[0]}",
            engine=engine,
            ins=[],
            outs=[],
            sync_info=mybir.SyncInfo(on_wait=[wait], on_update=[]),
        )

    for f in nc.m.functions:
        for blk in f.blocks:
            insts = blk.instructions
            out = []
            for inst in insts:
                si = inst.sync_info
                if si is not None and len(si.on_wait) > 1:
                    waits = list(si.on_wait)
                    for w in waits[:-1]:
                        out.append(mk(inst.engine, w))
                    inst.sync_info = mybir.SyncInfo(
                        on_wait=[waits[-1]], on_update=list(si.on_update)
                    )
                out.append(inst)
            insts[:] = out
    return nc


N_CORES = 8
B, T, C = 128, 256, 384
H, DH = 6, 64
BL = B // N_CORES  # batches per core
GB = 2  # batches per projection group (N = GB*T = 512 <= one PSUM bank fp32)
NG = BL // GB
BF16 = mybir.dt.bfloat16
FP32 = mybir.dt.float32
AFT = mybir.ActivationFunctionType
SCALE = DH**-0.5  # 0.125


def build_kernel() -> bass.Bass:
    nc = bass.Bass()
    xT = nc.dram_tensor("xT", [BL, C, T], BF16, kind="ExternalInput")
    wqt = nc.dram_tensor("wqt", [C, C], BF16, kind="ExternalInput")  # Wq.T [C, D]
    wkt = nc.dram_tensor("wkt", [C, C], BF16, kind="ExternalInput")
    wvt = nc.dram_tensor("wvt", [C, C], BF16, kind="ExternalInput")
    wot = nc.dram_tensor("wot", [C, C], BF16, kind="ExternalInput")  # Wo.T [D, C]
    y = nc.dram_tensor("y", [BL, T, C], FP32, kind="ExternalOutput")

    with tile.TileContext(nc) as tc:
        with (
            tc.tile_pool(name="const", bufs=1) as const,
            tc.tile_pool(name="xp", bufs=NG) as xp,
            tc.tile_pool(name="qkv", bufs=2) as qkv,
            tc.tile_pool(name="vp", bufs=4) as vp,
            tc.tile_pool(name="pp", bufs=4) as pp,
            tc.tile_pool(name="osb", bufs=3) as osb,
            tc.tile_pool(name="otp", bufs=2) as otp,
            tc.tile_pool(name="rsp", bufs=8) as rsp,
            tc.tile_pool(name="yp", bufs=3) as yp,
            tc.tile_pool(name="psBig", bufs=2, space="PSUM") as psBig,
            tc.tile_pool(name="psSt", bufs=2, space="PSUM") as psSt,
            tc.tile_pool(name="psM", bufs=2, space="PSUM") as psM,
            tc.tile_pool(name="psO", bufs=2, space="PSUM") as psO,
        ):
            ident = const.tile([128, 128], BF16)
            make_identity(nc, ident)
            # multiplicative 0/1 causal mask for PT [ts, tq], tq-packed as
            # [ts0 x tq0 | ts0 x tq1 | ts1 x tq1]: keep ts <= tq, so the
            # outer blocks are triangular (keep col >= partition), middle
            # block is all-ones
            mc2 = const.tile([128, 384], BF16)
            nc.gpsimd.memset(mc2, 1.0)
            for off in (0, 256):
                nc.gpsimd.affine_select(
                    out=mc2[:, off : off + 128],
                    in_=mc2[:, off : off + 128],
                    compare_op=mybir.AluOpType.is_ge,
                    fill=0.0,
                    base=0,
                    pattern=[[1, 128]],
                    channel_multiplier=-1,
                )

            w_sb = {}
            for name, dram in (("wq", wqt), ("wk", wkt), ("wv", wvt), ("wo", wot)):
                w = const.tile([128, 3, C], BF16, tag=name)
                nc.sync.dma_start(out=w, in_=dram.rearrange("(k p) d -> p k d", p=128))
                w_sb[name] = w

            # prefetch all xT tiles up front (24KB/partition total)
            xts = []
            for g in range(NG):
                xt = xp.tile([128, 3, GB, T], BF16, tag="x", name=f"xt{g}")
                for bi in range(GB):
                    nc.sync.dma_start(
                        out=xt[:, :, bi, :],
                        in_=xT[g * GB + bi].rearrange("(k p) t -> p k t", p=128),
                    )
                xts.append(xt)

            for g in range(NG):
                xt = xts[g]

                # ---- QT/KT for both batches: [D, b, T], N = GB*T ----
                qt = qkv.tile([128, 3, GB, T], BF16, tag="qt")
                kt = qkv.tile([128, 3, GB, T], BF16, tag="kt")
                for dst, wname in ((qt, "wq"), (kt, "wk")):
                    w = w_sb[wname]
                    for d in range(3):
                        ps = psBig.tile([128, GB * T], FP32, tag="big")
                        for k in range(3):
                            nc.tensor.matmul(
                                ps,
                                lhsT=w[:, k, d * 128 : (d + 1) * 128],
                                rhs=xt[:, k, :, :],
                                start=(k == 0),
                                stop=(k == 2),
                            )
                        nc.scalar.copy(dst[:, d, :, :], ps)

                # ---- V per batch: [ts, H, 65] with ones in col 64 ----
                v2s = []
                for bi in range(GB):
                    v2 = vp.tile([128, 2, H, 65], BF16, tag="v")
                    nc.gpsimd.memset(v2[:, :, :, 64:65], 1.0)
                    for t2 in range(2):
                        ps = psM.tile([128, C], FP32, tag="m", name="vps")
                        for k in range(3):
                            nc.tensor.matmul(
                                ps,
                                lhsT=xt[:, k, bi, t2 * 128 : (t2 + 1) * 128],
                                rhs=w_sb["wv"][:, k, :],
                                start=(k == 0),
                                stop=(k == 2),
                            )
                        nc.vector.tensor_copy(v2[:, t2, :, 0:64], ps)
                    v2s.append(v2)

                # ---- attention: flat pipeline over (batch, head) units,
                # AV/normalize lag scores/exp/mask by one unit ----
                units = [(bi, h) for bi in range(GB) for h in range(H)]
                pending = {}
                cur_osb = {}

                def emit_scores(i):
                    bi, h = units[i]
                    pair, doff = h // 2, (h % 2) * 64
                    qh = qt[doff : doff + 64, pair, bi, :]
                    kh = kt[doff : doff + 64, pair, bi, :]
                    st = psSt.tile([128, 384], FP32, tag="st")
                    nc.tensor.matmul(
                        st[:, 0:256], lhsT=kh[:, 0:128], rhs=qh, start=True, stop=True
                    )
                    nc.tensor.matmul(
                        st[:, 256:384],
                        lhsT=kh[:, 128:256],
                        rhs=qh[:, 128:256],
                        start=True,
                        stop=True,
                    )
                    pt_t = pp.tile([128, 384], BF16, tag="pt")
                    nc.scalar.activation(pt_t, st, AFT.Exp, scale=SCALE)
                    nc.vector.tensor_mul(pt_t, pt_t, mc2)
                    pending[i] = pt_t

                def emit_av(i):
                    bi, h = units[i]
                    pt_t = pending.pop(i)
                    v2 = v2s[bi]
                    o = psO.tile([128, 130], FP32, tag="o")
                    nc.tensor.matmul(
                        o[:, 0:65],
                        lhsT=pt_t[:, 0:128],
                        rhs=v2[:, 0, h, :],
                        start=True,
                        stop=True,
                    )
                    nc.tensor.matmul(
                        o[:, 65:130],
                        lhsT=pt_t[:, 128:256],
                        rhs=v2[:, 0, h, :],
                        start=True,
                        stop=False,
                    )
                    nc.tensor.matmul(
                        o[:, 65:130],
                        lhsT=pt_t[:, 256:384],
                        rhs=v2[:, 1, h, :],
                        start=False,
                        stop=True,
                    )
                    if h == 0:
                        cur_osb[bi] = osb.tile(
                            [128, 2, C], BF16, tag="osb", name="ob"
                        )
                    ob = cur_osb[bi]
                    for t2 in range(2):
                        rs = rsp.tile([128, 1], FP32, tag="rs")
                        nc.vector.reciprocal(rs, o[:, 65 * t2 + 64 : 65 * t2 + 65])
                        nc.vector.tensor_scalar_mul(
                            ob[:, t2, h * 64 : (h + 1) * 64],
                            o[:, 65 * t2 : 65 * t2 + 64],
                            rs,
                        )

                def emit_tail(bi):
                    # transpose O [tq, D] -> OT [D, tq], then y = OT^T @ WoT
                    ob = cur_osb[bi]
                    ot = otp.tile([128, 3, T], BF16, tag="ot")
                    for t2 in range(2):
                        tps = psM.tile([128, 384], BF16, tag="m", name="tps")
                        for db in range(3):
                            nc.tensor.transpose(
                                tps[:, db * 128 : (db + 1) * 128],
                                ob[:, t2, db * 128 : (db + 1) * 128],
                                ident,
                            )
                        nc.vector.tensor_copy(ot[:, :, t2 * 128 : (t2 + 1) * 128], tps)
                    for t2 in range(2):
                        ps = psBig.tile([128, GB * T], FP32, tag="big", name="ops")
                        for k in range(3):
                            nc.tensor.matmul(
                                ps[:, 0:C],
                                lhsT=ot[:, k, t2 * 128 : (t2 + 1) * 128],
                                rhs=w_sb["wo"][:, k, :],
                                start=(k == 0),
                                stop=(k == 2),
                            )
                        ys = yp.tile([128, C], FP32, tag="y", name="ys")
                        nc.scalar.copy(ys, ps[:, 0:C])
                        nc.sync.dma_start(
                            out=y[g * GB + bi, t2 * 128 : (t2 + 1) * 128, :],
                            in_=ys,
                        )

                for i in range(len(units) + 1):
                    if i < len(units):
                        emit_scores(i)
                    if i >= 1:
                        emit_av(i - 1)
                        bi, h = units[i - 1]
                        if h == H - 1:
                            emit_tail(bi)
    return nc


_NC = None


def _get_nc():
    global _NC
    if _NC is None:
        _NC = split_multi_waits(build_kernel())
    return _NC


def kernel(x, Wq, Wk, Wv, Wo, _trace=False):
    bf16 = ml_dtypes.bfloat16
    wq_t = np.ascontiguousarray(Wq.T).astype(bf16)
    wk_t = np.ascontiguousarray(Wk.T).astype(bf16)
    wv_t = np.ascontiguousarray(Wv.T).astype(bf16)
    wo_t = np.ascontiguousarray(Wo.T).astype(bf16)
    in_maps = []
    for i in range(N_CORES):
        xs = x[i * BL : (i + 1) * BL]  # [BL, T, C]
        xs_t = np.ascontiguousarray(xs.transpose(0, 2, 1)).astype(bf16)
        in_maps.append(
            {"xT": xs_t, "wqt": wq_t, "wkt": wk_t, "wvt": wv_t, "wot": wo_t}
        )
    res = run_bass_kernel_spmd(
        _get_nc(), in_maps, list(range(N_CORES)), trace=_trace
    )
    out = np.concatenate([r["y"] for r in res.results], axis=0)
    if _trace:
        return out.astype(np.float32), res
    return out.astype(np.float32)


# revision 7
# speedup vs baseline: 2.1765x; 1.3433x over previous
"""Causal multi-head attention (B=128, T=256, C=384, H=6, Dh=64) on 8 TRN2
NeuronCores, data-parallel over batch (16 batches per core, no collectives).

Layout strategy per core (v2 — transposed-scores):
  - host pre-transposes x to xT [b, C, T] and casts activations/weights to bf16
  - QT/KT computed as [D, T] (Dh on partitions)
  - scores are computed TRANSPOSED: ST[ts, tq] = KT_h^T @ QT_h, so
    PT = exp(ST)*mask feeds the AV matmul directly as the stationary
    operand — no PE transposes of P at all
  - V stored per head with an appended ones column ([ts, 65]); the AV
    matmul out = PT^T @ [V_h | 1] lands O[tq, 0:64] AND the softmax
    denominators in col 64 of the same PSUM tile — row sums are free
  - normalization is a per-partition tensor_scalar_mul fused into the
    PSUM->SBUF move (exp is never max-subtracted; scores are O(30) so
    fp32 exp cannot overflow)
  - O [tq, D] is transposed on the PE (6 [128,128] blocks per batch) to
    OT for the output projection, which consumes OT as stationary
  - attention is software-pipelined one head ahead so the PE never waits
    on the scalar(exp)/vector(mask) stages of the same head
"""

import sys

sys.path.insert(0, "/opt/trn_rl_repo")

import numpy as np
import ml_dtypes

import concourse.bass as bass
import concourse.tile as tile
from concourse import mybir
from concourse.bass_utils import run_bass_kernel_spmd
from concourse.masks import make_identity


def split_multi_waits(nc):
    """This walrus build accepts at most one sync-wait command per
    instruction; hoist extra waits into standalone InstEventSemaphore
    instructions on the same engine queue (queue waits run in order before
    the original instruction, so semantics are preserved)."""
    ctr = [0]

    def mk(engine, wait):
        ctr[0] += 1
        return mybir.InstEventSemaphore(
            name=f"WSPLIT-{ctr[0]}",
            engine=engine,
            ins=[],
            outs=[],
            sync_info=mybir.SyncInfo(on_wait=[wait], on_update=[]),
        )

    for f in nc.m.functions:
        for blk in f.blocks:
            insts = blk.instructions
            out = []
            for inst in insts:
                si = inst.sync_info
                if si is not None and len(si.on_wait) > 1:
                    waits = list(si.on_wait)
                    for w in waits[:-1]:
                        out.append(mk(inst.engine, w))
                    inst.sync_info = mybir.SyncInfo(
                        on_wait=[waits[-1]], on_update=list(si.on_update)
                    )
                out.append(inst)
            insts[:] = out
    return nc


N_CORES = 8
B, T, C = 128, 256, 384
H, DH = 6, 64
BL = B // N_CORES  # batches per core
GB = 2  # batches per projection group (N = GB*T = 512 <= one PSUM bank fp32)
NG = BL // GB
BF16 = mybir.dt.bfloat16
FP32 = mybir.dt.float32
AFT = mybir.ActivationFunctionType
SCALE = DH**-0.5  # 0.125


def build_kernel() -> bass.Bass:
    nc = bass.Bass()
    xT = nc.dram_tensor("xT", [BL, C, T], BF16, kind="ExternalInput")
    wqt = nc.dram_tensor("wqt", [C, C], BF16, kind="ExternalInput")  # Wq.T [C, D]
    wkt = nc.dram_tensor("wkt", [C, C], BF16, kind="ExternalInput")
    wvt = nc.dram_tensor("wvt", [C, C], BF16, kind="ExternalInput")
    wot = nc.dram_tensor("wot", [C, C], BF16, kind="ExternalInput")  # Wo.T [D, C]
    y = nc.dram_tensor("y", [BL, T, C], FP32, kind="ExternalOutput")

    with tile.TileContext(nc) as tc:
        with (
            tc.tile_pool(name="const", bufs=1) as const,
            tc.tile_pool(name="xp", bufs=NG) as xp,
            tc.tile_pool(name="qkv", bufs=2) as qkv,
            tc.tile_pool(name="vp", bufs=4) as vp,
            tc.tile_pool(name="pp", bufs=4) as pp,
            tc.tile_pool(name="osb", bufs=3) as osb,
            tc.tile_pool(name="otp", bufs=2) as otp,
            tc.tile_pool(name="rsp", bufs=8) as rsp,
            tc.tile_pool(name="yp", bufs=3) as yp,
            tc.tile_pool(name="psBig", bufs=2, space="PSUM") as psBig,
            tc.tile_pool(name="psSt", bufs=2, space="PSUM") as psSt,
            tc.tile_pool(name="psM", bufs=2, space="PSUM") as psM,
            tc.tile_pool(name="psO", bufs=2, space="PSUM") as psO,
        ):
            ident = const.tile([128, 128], BF16)
            make_identity(nc, ident)
            # multiplicative 0/1 causal mask for PT [ts, tq], tq-packed as
            # [ts0 x tq0 | ts0 x tq1 | ts1 x tq1]: keep ts <= tq, so the
            # outer blocks are triangular (keep col >= partition), middle
            # block is all-ones
            mc2 = const.tile([128, 384], BF16)
            nc.gpsimd.memset(mc2, 1.0)
            for off in (0, 256):
                nc.gpsimd.affine_select(
                    out=mc2[:, off : off + 128],
                    in_=mc2[:, off : off + 128],
                    compare_op=mybir.AluOpType.is_ge,
                    fill=0.0,
                    base=0,
                    pattern=[[1, 128]],
                    channel_multiplier=-1,
                )

            w_sb = {}
            for name, dram in (("wq", wqt), ("wk", wkt), ("wv", wvt), ("wo", wot)):
                w = const.tile([128, 3, C], BF16, tag=name)
                nc.sync.dma_start(out=w, in_=dram.rearrange("(k p) d -> p k d", p=128))
                w_sb[name] = w

            # prefetch all xT tiles up front (24KB/partition total)
            xts = []
            for g in range(NG):
                xt = xp.tile([128, 3, GB, T], BF16, tag="x", name=f"xt{g}")
                for bi in range(GB):
                    nc.sync.dma_start(
                        out=xt[:, :, bi, :],
                        in_=xT[g * GB + bi].rearrange("(k p) t -> p k t", p=128),
                    )
                xts.append(xt)

            # per-group projection pre-work, chunked as a generator so it can
            # be interleaved into the previous group's attention units
            group_state = {}

            def gen_prework(g):
                xt = xts[g]
                qtkt = {}
                for dname, wname in (("qt", "wq"), ("kt", "wk")):
                    t = qkv.tile(
                        [128, 3, GB, T], BF16, tag=dname, name=f"{dname}{g}"
                    )
                    qtkt[dname] = t
                    w = w_sb[wname]
                    for d in range(3):
                        ps = psBig.tile(
                            [128, GB * T], FP32, tag="big", name="qkps"
                        )
                        for k in range(3):
                            nc.tensor.matmul(
                                ps,
                                lhsT=w[:, k, d * 128 : (d + 1) * 128],
                                rhs=xt[:, k, :, :],
                                start=(k == 0),
                                stop=(k == 2),
                            )
                        nc.scalar.copy(t[:, d, :, :], ps)
                        yield
                v2s = []
                for bi in range(GB):
                    v2 = vp.tile([128, 2, H, 65], BF16, tag="v", name="v2")
                    nc.gpsimd.memset(v2[:, :, :, 64:65], 1.0)
                    for t2 in range(2):
                        ps = psM.tile([128, C], FP32, tag="m", name="vps")
                        for k in range(3):
                            nc.tensor.matmul(
                                ps,
                                lhsT=xt[:, k, bi, t2 * 128 : (t2 + 1) * 128],
                                rhs=w_sb["wv"][:, k, :],
                                start=(k == 0),
                                stop=(k == 2),
                            )
                        nc.vector.tensor_copy(v2[:, t2, :, 0:64], ps)
                        yield
                    v2s.append(v2)
                group_state[g] = (qtkt["qt"], qtkt["kt"], v2s)

            pending = {}
            cur_osb = {}

            def emit_scores(u):
                g, bi, h = u
                qt, kt, _ = group_state[g]
                pair, doff = h // 2, (h % 2) * 64
                qh = qt[doff : doff + 64, pair, bi, :]
                kh = kt[doff : doff + 64, pair, bi, :]
                st = psSt.tile([128, 384], FP32, tag="st", name="st")
                nc.tensor.matmul(
                    st[:, 0:256], lhsT=kh[:, 0:128], rhs=qh, start=True, stop=True
                )
                nc.tensor.matmul(
                    st[:, 256:384],
                    lhsT=kh[:, 128:256],
                    rhs=qh[:, 128:256],
                    start=True,
                    stop=True,
                )
                pt_t = pp.tile([128, 384], BF16, tag="pt", name="pt")
                nc.scalar.activation(pt_t, st, AFT.Exp, scale=SCALE)
                nc.gpsimd.tensor_mul(pt_t, pt_t, mc2)
                pending[u] = pt_t

            def emit_av(u):
                g, bi, h = u
                _, _, v2s = group_state[g]
                pt_t = pending.pop(u)
                v2 = v2s[bi]
                o = psO.tile([128, 2, 65], FP32, tag="o", name="o")
                nc.tensor.matmul(
                    o[:, 0, :],
                    lhsT=pt_t[:, 0:128],
                    rhs=v2[:, 0, h, :],
                    start=True,
                    stop=True,
                )
                nc.tensor.matmul(
                    o[:, 1, :],
                    lhsT=pt_t[:, 128:256],
                    rhs=v2[:, 0, h, :],
                    start=True,
                    stop=False,
                )
                nc.tensor.matmul(
                    o[:, 1, :],
                    lhsT=pt_t[:, 256:384],
                    rhs=v2[:, 1, h, :],
                    start=False,
                    stop=True,
                )
                if h == 0:
                    cur_osb[bi] = osb.tile([128, 2, C], BF16, tag="osb", name="ob")
                ob = cur_osb[bi]
                rs = rsp.tile([128, 2], FP32, tag="rs", name="rs")
                nc.vector.reciprocal(rs, o[:, :, 64:65])
                for t2 in range(2):
                    nc.vector.tensor_scalar_mul(
                        ob[:, t2, h * 64 : (h + 1) * 64],
                        o[:, t2, 0:64],
                        rs[:, t2 : t2 + 1],
                    )

            def emit_tail(g, bi):
                # transpose O [tq, D] -> OT [D, tq], then y = OT^T @ WoT
                ob = cur_osb[bi]
                ot = otp.tile([128, 3, T], BF16, tag="ot", name="ot")
                for t2 in range(2):
                    tps = psM.tile([128, 384], BF16, tag="m", name="tps")
                    for db in range(3):
                        nc.tensor.transpose(
                            tps[:, db * 128 : (db + 1) * 128],
                            ob[:, t2, db * 128 : (db + 1) * 128],
                            ident,
                        )
                    nc.vector.tensor_copy(ot[:, :, t2 * 128 : (t2 + 1) * 128], tps)
                for t2 in range(2):
                    ps = psBig.tile([128, GB * T], FP32, tag="big", name="ops")
                    for k in range(3):
                        nc.tensor.matmul(
                            ps[:, 0:C],
                            lhsT=ot[:, k, t2 * 128 : (t2 + 1) * 128],
                            rhs=w_sb["wo"][:, k, :],
                            start=(k == 0),
                            stop=(k == 2),
                        )
                    ys = yp.tile([128, C], FP32, tag="y", name="ys")
                    nc.scalar.copy(ys, ps[:, 0:C])
                    nc.sync.dma_start(
                        out=y[g * GB + bi, t2 * 128 : (t2 + 1) * 128, :],
                        in_=ys,
                    )

            gens = [gen_prework(g) for g in range(NG)]
            for _ in gens[0]:  # group 0 pre-work as prologue
                pass
            units = [
                (g, bi, h) for g in range(NG) for bi in range(GB) for h in range(H)
            ]
            UPG = GB * H  # units per group
            for i in range(len(units) + 1):
                if i < len(units):
                    emit_scores(units[i])
                if i >= 1:
                    u = units[i - 1]
                    emit_av(u)
                    # interleave next group's projection chunk after each AV
                    gnext = u[0] + 1
                    if gnext < NG:
                        next(gens[gnext], None)
                    if u[2] == H - 1:
                        emit_tail(u[0], u[1])
    return nc


_NC = None


def _get_nc():
    global _NC
    if _NC is None:
        _NC = split_multi_waits(build_kernel())
    return _NC


def kernel(x, Wq, Wk, Wv, Wo, _trace=False):
    bf16 = ml_dtypes.bfloat16
    wq_t = np.ascontiguousarray(Wq.T).astype(bf16)
    wk_t = np.ascontiguousarray(Wk.T).astype(bf16)
    wv_t = np.ascontiguousarray(Wv.T).astype(bf16)
    wo_t = np.ascontiguousarray(Wo.T).astype(bf16)
    in_maps = []
    for i in range(N_CORES):
        xs = x[i * BL : (i + 1) * BL]  # [BL, T, C]
        xs_t = np.ascontiguousarray(xs.transpose(0, 2, 1)).astype(bf16)
        in_maps.append(
            {"xT": xs_t, "wqt": wq_t, "wkt": wk_t, "wvt": wv_t, "wot": wo_t}
        )
    res = run_bass_kernel_spmd(
        _get_nc(), in_maps, list(range(N_CORES)), trace=_trace
    )
    out = np.concatenate([r["y"] for r in res.results], axis=0)
    if _trace:
        return out.astype(np.float32), res
    return out.astype(np.float32)
